# revision 5
# baseline (speedup 1.0000x reference)
import sys

if "/opt/trn_rl_repo" not in sys.path:
    sys.path.insert(0, "/opt/trn_rl_repo")

import numpy as np

N_G = 90
NP1 = 91  # N_G + 1 (epsilon-padded graph order)
NB_LABELS = 10
NB_EDGE_LABELS = 2
SINKHORN_ITERS = 10
N_CORES = 8
QUAD_TERMS = 4  # packed (P,Q) pairs of the separable quadratic form


def _build_nc():
    """Build + compile the single-core Bass/Tile program (run SPMD on 8 cores).

    GED for one graph pair.  The 8281x8281 cost matrix C is never formed:
    every Kronecker block of C is separable, so  v' F v = sum_t S.(P_t S Q_t)
    with 91x91 factors.  The device computes:
      Dg   = L1 @ NC @ L2' + insdel border        (node-assignment costs)
      S0   = exp(-0.5 Dg)
      S    = 10 Sinkhorn iterations (row/col normalize, eps row/col skipped)
      M    = Dg + sum_t P_t @ S @ Q_t             (0.5 weight folded into P_t)
      ged  = sum S.M - sum diagF.(S.S)            (diagF = packed outer products)
    """
    import concourse.bass as bass
    import concourse.tile as tile
    from concourse import bacc, mybir

    f32 = mybir.dt.float32
    AX = mybir.AxisListType.X
    ALU = mybir.AluOpType
    ACTF = mybir.ActivationFunctionType

    nc = bacc.Bacc(None, debug=False)

    dPT = nc.declare_dram_parameter("PT", [NP1, NP1 * QUAD_TERMS], f32, isOutput=False)
    dQ = nc.declare_dram_parameter("Q", [NP1, NP1 * QUAD_TERMS], f32, isOutput=False)
    dL1T = nc.declare_dram_parameter("L1T", [NB_LABELS, NP1], f32, isOutput=False)
    dL2T = nc.declare_dram_parameter("L2T", [NB_LABELS, NP1], f32, isOutput=False)
    dNC = nc.declare_dram_parameter("NC", [NB_LABELS, NB_LABELS], f32, isOutput=False)
    dU2 = nc.declare_dram_parameter("U2", [2, NP1], f32, isOutput=False)
    dW2 = nc.declare_dram_parameter("W2", [2, NP1], f32, isOutput=False)
    dU4 = nc.declare_dram_parameter("U4", [4, NP1], f32, isOutput=False)
    dW4 = nc.declare_dram_parameter("W4", [4, NP1], f32, isOutput=False)
    dI = nc.declare_dram_parameter("ident", [NP1, NP1], f32, isOutput=False)
    dOnes = nc.declare_dram_parameter("ones", [NP1, 1], f32, isOutput=False)
    dOut = nc.declare_dram_parameter("out", [1, 1], f32, isOutput=True)

    with tile.TileContext(nc) as tc:
        with (
            tc.tile_pool(name="const", bufs=1) as cpool,
            tc.tile_pool(name="sk", bufs=2) as sk,
            tc.tile_pool(name="vec", bufs=2) as vec,
            tc.tile_pool(name="fin", bufs=1) as fin,
            tc.tile_pool(name="ps_dg", bufs=1, space=bass.MemorySpace.PSUM) as ps_dg,
            tc.tile_pool(name="ps_sk", bufs=2, space=bass.MemorySpace.PSUM) as ps_sk,
            tc.tile_pool(name="ps_mm", bufs=1, space=bass.MemorySpace.PSUM) as ps_mm,
        ):
            # ---- load inputs ----
            tPT = cpool.tile([NP1, NP1 * QUAD_TERMS], f32)
            tQ = cpool.tile([NP1, NP1 * QUAD_TERMS], f32)
            tL1T = cpool.tile([NB_LABELS, NP1], f32)
            tL2T = cpool.tile([NB_LABELS, NP1], f32)
            tNC = cpool.tile([NB_LABELS, NB_LABELS], f32)
            tU2 = cpool.tile([2, NP1], f32)
            tW2 = cpool.tile([2, NP1], f32)
            tU4 = cpool.tile([4, NP1], f32)
            tW4 = cpool.tile([4, NP1], f32)
            tI = cpool.tile([NP1, NP1], f32)
            tOnes = cpool.tile([NP1, 1], f32)
            for t, d in (
                (tL1T, dL1T), (tL2T, dL2T), (tNC, dNC), (tU2, dU2), (tW2, dW2),
                (tI, dI), (tOnes, dOnes), (tU4, dU4), (tW4, dW4), (tPT, dPT),
                (tQ, dQ),
            ):
                nc.sync.dma_start(t[:], d[:])

            # persistent reciprocal normalizers; element [90] stays 1.0
            # (epsilon row/col is not normalized)
            rden = fin.tile([N_G, 1], f32, tag="rden")
            cden = fin.tile([N_G, 1], f32, tag="cden")
            rrec = fin.tile([NP1, 1], f32, tag="rrec")
            crec = fin.tile([NP1, 1], f32, tag="crec")
            nc.vector.memset(rrec[:], 1.0)
            nc.vector.memset(crec[:], 1.0)

            # ---- Dg = L1 @ NC @ L2' + border ----
            p_xt = ps_dg.tile([NB_LABELS, NP1], f32, tag="xt")
            nc.tensor.matmul(p_xt[:], tNC[:], tL1T[:], start=True, stop=True)
            s_xt = fin.tile([NB_LABELS, NP1], f32, tag="xt_s")
            nc.scalar.copy(s_xt[:], p_xt[:])
            p_dg = ps_dg.tile([NP1, NP1], f32, tag="dg")
            nc.tensor.matmul(p_dg[:], s_xt[:], tL2T[:], start=True, stop=False)
            nc.tensor.matmul(p_dg[:], tU2[:], tW2[:], start=False, stop=True)

            # ---- S0 = exp(-0.5 Dg) ----
            s_cur = sk.tile([NP1, NP1], f32, tag="s_sb")
            nc.scalar.activation(s_cur[:], p_dg[:], ACTF.Exp, scale=-0.5)

            # ---- Sinkhorn: row normalize, transpose, col normalize, transpose ----
            cur = s_cur  # SBUF on iter 0, PSUM afterwards
            last_T = None
            for _ in range(SINKHORN_ITERS):
                nc.vector.reduce_sum(rden[:], cur[:N_G, :], axis=AX)
                nc.vector.reciprocal(rrec[:N_G, :], rden[:])
                s_n = sk.tile([NP1, NP1], f32, tag="s_sb")
                nc.vector.tensor_scalar_mul(s_n[:], cur[:], rrec[:])
                p_t = ps_sk.tile([NP1, NP1], f32, tag="s_ps")
                nc.tensor.transpose(p_t[:], s_n[:], tI[:])

                nc.vector.reduce_sum(cden[:], p_t[:N_G, :], axis=AX)
                nc.vector.reciprocal(crec[:N_G, :], cden[:])
                st_n = sk.tile([NP1, NP1], f32, tag="s_sb")
                nc.vector.tensor_scalar_mul(st_n[:], p_t[:], crec[:])
                p_s = ps_sk.tile([NP1, NP1], f32, tag="s_ps")
                nc.tensor.transpose(p_s[:], st_n[:], tI[:])
                cur = p_s
                last_T = st_n
            # cur = S (PSUM), last_T = S' (SBUF)

            # ---- quadratic form ----
            p_vb = ps_mm.tile([NP1, NP1 * QUAD_TERMS], f32, tag="vb")
            nc.tensor.matmul(p_vb[:], last_T[:], tQ[:], start=True, stop=True)
            s_vb = fin.tile([NP1, NP1 * QUAD_TERMS], f32, tag="vb_s")
            nc.scalar.copy(s_vb[:], p_vb[:])

            p_m = ps_dg.tile([NP1, NP1], f32, tag="m")
            nc.tensor.matmul(p_m[:], s_xt[:], tL2T[:], start=True, stop=False)
            nc.tensor.matmul(p_m[:], tU2[:], tW2[:], start=False, stop=False)
            for t in range(QUAD_TERMS):
                sl = slice(t * NP1, (t + 1) * NP1)
                nc.tensor.matmul(
                    p_m[:], tPT[:, sl], s_vb[:, sl],
                    start=False, stop=(t == QUAD_TERMS - 1))

            # diagF via packed outer products (independent; off critical path)
            p_df = ps_mm.tile([NP1, NP1], f32, tag="df")
            nc.tensor.matmul(p_df[:], tU4[:], tW4[:], start=True, stop=True)
            s_df = fin.tile([NP1, NP1], f32, tag="df_s")
            nc.scalar.copy(s_df[:], p_df[:])

            # ---- ged = sum S.(M - diagF.S) ----
            t1 = vec.tile([NP1, NP1], f32, tag="t1")
            nc.vector.tensor_mul(t1[:], s_df[:], cur[:])
            t2 = vec.tile([NP1, NP1], f32, tag="t2")
            nc.vector.tensor_sub(t2[:], p_m[:], t1[:])
            t3 = vec.tile([NP1, NP1], f32, tag="t3")
            nc.vector.tensor_mul(t3[:], t2[:], cur[:])
            rowsum = fin.tile([NP1, 1], f32, tag="rowsum")
            nc.vector.reduce_sum(rowsum[:], t3[:], axis=AX)

            p_ged = ps_mm.tile([1, 1], f32, tag="ged")
            nc.tensor.matmul(p_ged[:], rowsum[:], tOnes[:], start=True, stop=True)
            s_out = fin.tile([1, 1], f32, tag="out_s")
            nc.scalar.copy(s_out[:], p_ged[:])
            nc.sync.dma_start(dOut[:], s_out[:])

    nc.compile()
    return nc


def _prep_inputs(adjacenceMatrix, labels, node_weighs, edge_weighs):
    """Host-side encoding of the raw integer/float inputs into the dense f32
    operands of the device program (layout prep only; all O(n^2) compute —
    Sinkhorn, matmuls, reductions — happens on device)."""
    f = np.float32
    nw = np.maximum(np.asarray(node_weighs, dtype=f), 0.0)
    ew = np.maximum(np.asarray(edge_weighs, dtype=f), 0.0)
    iu, ju = np.triu_indices(NB_LABELS, k=1)
    NC = np.zeros((NB_LABELS, NB_LABELS), f)
    NC[iu, ju] = nw[:-1]
    NC = NC + NC.T
    node_ins_del = nw[-1]
    ce0 = ew[0]      # edge substitution cost (2 labels -> single off-diag)
    e_id = ew[-1]    # edge insertion/deletion cost

    adj = np.asarray(adjacenceMatrix)
    A1 = np.zeros((NP1, NP1), np.int64)
    A1[:N_G, :N_G] = adj[0][: N_G * N_G].reshape(N_G, N_G)
    A2 = np.zeros((NP1, NP1), np.int64)
    A2[:N_G, :N_G] = adj[1][: N_G * N_G].reshape(N_G, N_G)
    Ab1 = (A1 != 0).astype(f)
    Ab2 = (A2 != 0).astype(f)
    oh1 = [(A1 == a).astype(f) for a in (1, 2)]
    oh2 = [(A2 == a).astype(f) for a in (1, 2)]

    lab = np.asarray(labels)
    L1T = np.zeros((NB_LABELS, NP1), f)
    L1T[lab[0][:N_G].astype(np.int64), np.arange(N_G)] = 1.0
    L2T = np.zeros((NB_LABELS, NP1), f)
    L2T[lab[1][:N_G].astype(np.int64), np.arange(N_G)] = 1.0

    J = np.ones((NP1, NP1), f)
    # quadratic-form pairs (P_t carries the 0.5 of ged = 0.5 v'Dm v + c.v)
    PT = np.concatenate([
        0.5 * e_id * Ab1.T,
        0.5 * e_id * J - e_id * Ab1.T,
        0.5 * ce0 * oh1[0].T,
        0.5 * ce0 * oh1[1].T,
    ], axis=1)
    Q = np.concatenate([J, Ab2, oh2[1], oh2[0]], axis=1)

    # node ins/del border of Dg: a = 1 on real nodes, b = eps indicator
    a = np.ones(NP1, f)
    a[N_G] = 0.0
    b = np.zeros(NP1, f)
    b[N_G] = 1.0
    U2 = np.stack([node_ins_del * a, node_ins_del * b])
    W2 = np.stack([b, a])

    d1 = np.diag(Ab1).astype(f)
    d2 = np.diag(Ab2).astype(f)
    dg1 = [np.diag(o).astype(f) for o in oh1]
    dg2 = [np.diag(o).astype(f) for o in oh2]
    U4 = np.stack([
        0.5 * e_id * d1,
        0.5 * e_id * np.ones(NP1, f) - e_id * d1,
        0.5 * ce0 * dg1[0],
        0.5 * ce0 * dg1[1],
    ])
    W4 = np.stack([np.ones(NP1, f), d2, dg2[1], dg2[0]])

    return {
        "PT": np.ascontiguousarray(PT),
        "Q": np.ascontiguousarray(Q),
        "L1T": L1T, "L2T": L2T, "NC": NC,
        "U2": np.ascontiguousarray(U2), "W2": np.ascontiguousarray(W2),
        "U4": np.ascontiguousarray(U4), "W4": np.ascontiguousarray(W4),
        "ident": np.eye(NP1, dtype=f),
        "ones": np.ones((NP1, 1), f),
    }


_NC = None


def _get_nc():
    global _NC
    if _NC is None:
        _NC = _build_nc()
    return _NC


def kernel(graph, adjacenceMatrix, graphCard, labels, node_weighs, edge_weighs):
    from concourse.bass_utils import run_bass_kernel_spmd

    in_map = _prep_inputs(adjacenceMatrix, labels, node_weighs, edge_weighs)
    res = run_bass_kernel_spmd(
        _get_nc(), [in_map] * N_CORES, core_ids=list(range(N_CORES)))
    return np.float32(res.results[0]["out"][0, 0])


# revision 6
# speedup vs baseline: 1.3088x; 1.3088x over previous
import sys

if "/opt/trn_rl_repo" not in sys.path:
    sys.path.insert(0, "/opt/trn_rl_repo")

import numpy as np

N_G = 90
NP1 = 91   # N_G + 1 (epsilon-padded graph order)
NP2 = 92   # NP1 + 1 (transpose output carries a sums column)
NB_LABELS = 10
SINKHORN_ITERS = 10
N_CORES = 8


def _build_nc():
    """Single-core Bass/Tile program, run replicated SPMD on 8 cores.

    GED of one graph pair.  The 8281x8281 cost matrix C is never formed:
    each Kronecker block is separable, so v'Fv = sum_t S.(P_t S Q_t) with
    91x91 factors.  Device pipeline (f32 PSUM accumulation throughout):
      Dg   = L1 @ NCL2G + insdel border                   (f32 matmuls)
      S0   = exp(-0.5 Dg)                                 (bf16)
      S    = 10 Sinkhorn iterations; each transpose is a bf16 matmul with
             rhs [I|1] so the new frame's row sums ride in column 91
      M    = sum_t P_t S Q_t  with exact-bf16 {0,.5,1} factors, split into
             an edgeInsDel group and an edge-cost group; runtime weights
             applied later on DVE in f32 (keeps bf16 operands exact)
      ged  = sum S.(eid*Meid + ce0*Mce0 + Dg - diagF.S)
    """
    import concourse.bass as bass
    import concourse.tile as tile
    from concourse import bacc, mybir

    f32 = mybir.dt.float32
    bf16 = mybir.dt.bfloat16
    AX = mybir.AxisListType.X
    ALU = mybir.AluOpType
    ACTF = mybir.ActivationFunctionType

    nc = bacc.Bacc(None, debug=False)

    dL1T = nc.declare_dram_parameter("L1T", [NB_LABELS, NP1], f32, isOutput=False)
    dNCL2 = nc.declare_dram_parameter("NCL2", [NB_LABELS, NP1], f32, isOutput=False)
    dU2 = nc.declare_dram_parameter("U2", [2, NP1], f32, isOutput=False)
    dW2 = nc.declare_dram_parameter("W2", [2, NP1], f32, isOutput=False)
    dIp = nc.declare_dram_parameter("Ipad", [NP1, NP2], bf16, isOutput=False)
    dPTe = nc.declare_dram_parameter("PTe", [NP1, 2 * NP1], bf16, isOutput=False)
    dPTc = nc.declare_dram_parameter("PTc", [NP1, 2 * NP1], bf16, isOutput=False)
    dQp = nc.declare_dram_parameter("Qp", [NP1, 4 * NP1], bf16, isOutput=False)
    dU4 = nc.declare_dram_parameter("U4", [4, NP1], bf16, isOutput=False)
    dW4 = nc.declare_dram_parameter("W4", [4, NP1], bf16, isOutput=False)
    dOnes = nc.declare_dram_parameter("ones", [NP1, 1], f32, isOutput=False)
    dWv = nc.declare_dram_parameter("wv", [NP1, 2], f32, isOutput=False)
    dOut = nc.declare_dram_parameter("out", [1, 1], f32, isOutput=True)

    with tile.TileContext(nc) as tc:
        with (
            tc.tile_pool(name="const", bufs=1) as cpool,
            tc.tile_pool(name="sk", bufs=2) as sk,
            tc.tile_pool(name="vec", bufs=1) as vec,
            tc.tile_pool(name="ps_dg", bufs=1, space=bass.MemorySpace.PSUM) as ps_dg,
            tc.tile_pool(name="ps_sk", bufs=2, space=bass.MemorySpace.PSUM) as ps_sk,
            tc.tile_pool(name="ps_mm", bufs=1, space=bass.MemorySpace.PSUM) as ps_mm,
        ):
            # ---- inputs; early tensors on the sync queue, late on gpsimd ----
            tL1T = cpool.tile([NB_LABELS, NP1], f32)
            tNCL2 = cpool.tile([NB_LABELS, NP1], f32)
            tU2 = cpool.tile([2, NP1], f32)
            tW2 = cpool.tile([2, NP1], f32)
            tIp = cpool.tile([NP1, NP2], bf16)
            for t, d in ((tL1T, dL1T), (tNCL2, dNCL2), (tU2, dU2), (tW2, dW2),
                         (tIp, dIp)):
                nc.sync.dma_start(t[:], d[:])
            tPTe = cpool.tile([NP1, 2 * NP1], bf16)
            tPTc = cpool.tile([NP1, 2 * NP1], bf16)
            tQp = cpool.tile([NP1, 4 * NP1], bf16)
            tU4 = cpool.tile([4, NP1], bf16)
            tW4 = cpool.tile([4, NP1], bf16)
            tOnes = cpool.tile([NP1, 1], f32)
            tWv = cpool.tile([NP1, 2], f32)
            for t, d in ((tQp, dQp), (tPTe, dPTe), (tPTc, dPTc), (tU4, dU4),
                         (tW4, dW4), (tOnes, dOnes), (tWv, dWv)):
                nc.gpsimd.dma_start(t[:], d[:])

            # persistent reciprocal normalizer; [90] stays 1.0 (eps not normed)
            rden = vec.tile([N_G, 1], f32, tag="rden")
            rrec = vec.tile([NP1, 1], f32, tag="rrec")
            nc.vector.memset(rrec[:], 1.0)

            # ---- Dg = L1 @ NCL2G + border ----
            p_dg = ps_dg.tile([NP1, NP1], f32, tag="dg")
            nc.tensor.matmul(p_dg[:], tL1T[:], tNCL2[:], start=True, stop=False)
            nc.tensor.matmul(p_dg[:], tU2[:], tW2[:], start=False, stop=True)

            # ---- S0 = exp(-0.5 Dg), bf16 ----
            s_cur = sk.tile([NP1, NP1], bf16, tag="s_sb")
            nc.scalar.activation(s_cur[:], p_dg[:], ACTF.Exp, scale=-0.5)
            nc.vector.reduce_sum(rden[:], s_cur[:N_G, :], axis=AX)

            # ---- Sinkhorn: 20 half-steps of normalize + fused transpose ----
            # p_h = frame_n' @ [I|1]: cols :91 = transposed frame, col 91 =
            # the new frame's row sums (only rows :90 are ever normalized).
            p_h = None
            last_T = None
            for h in range(2 * SINKHORN_ITERS):
                if h == 0:
                    nc.vector.reciprocal(rrec[:N_G, :], rden[:])
                else:
                    nc.vector.reciprocal(rrec[:N_G, :], p_h[:N_G, NP1:NP2])
                s_n = sk.tile([NP1, NP1], bf16, tag="s_sb")
                src = s_cur[:] if h == 0 else p_h[:, :NP1]
                nc.vector.tensor_scalar_mul(s_n[:], src, rrec[:])
                p_h = ps_sk.tile([NP1, NP2], f32, tag="s_ps")
                nc.tensor.matmul(p_h[:], s_n[:], tIp[:], start=True, stop=True)
                last_T = s_n
            cur = p_h  # V in PSUM (col 91 junk); last_T = V' in SBUF, bf16

            # ---- quadratic form, exact bf16 factors ----
            p_vb = ps_mm.tile([NP1, 4 * NP1], f32, tag="vb")
            nc.tensor.matmul(p_vb[:], last_T[:], tQp[:], start=True, stop=True)
            s_vb = cpool.tile([NP1, 4 * NP1], bf16, tag="vb_s")
            nc.scalar.copy(s_vb[:], p_vb[:])
            p_me = ps_dg.tile([NP1, NP1], f32, tag="meid")
            nc.tensor.matmul(p_me[:], tPTe[:, :NP1], s_vb[:, :NP1],
                             start=True, stop=False)
            nc.tensor.matmul(p_me[:], tPTe[:, NP1:], s_vb[:, NP1:2 * NP1],
                             start=False, stop=True)
            p_mc = ps_dg.tile([NP1, NP1], f32, tag="mce")
            nc.tensor.matmul(p_mc[:], tPTc[:, :NP1], s_vb[:, 2 * NP1:3 * NP1],
                             start=True, stop=False)
            nc.tensor.matmul(p_mc[:], tPTc[:, NP1:], s_vb[:, 3 * NP1:],
                             start=False, stop=True)

            # diagF (host-folded weights); off critical path
            p_df = ps_mm.tile([NP1, NP1], f32, tag="df")
            nc.tensor.matmul(p_df[:], tU4[:], tW4[:], start=True, stop=True)
            s_df = cpool.tile([NP1, NP1], f32, tag="df_s")
            nc.scalar.copy(s_df[:], p_df[:])

            # ---- ged = sum V.(eid*Meid + ce0*Mce0 + Dg - diagF.V) ----
            t1 = vec.tile([NP1, NP1], f32, tag="t1")
            nc.vector.tensor_mul(t1[:], s_df[:], cur[:, :NP1])
            m1 = vec.tile([NP1, NP1], f32, tag="m1")
            nc.vector.tensor_scalar_mul(m1[:], p_mc[:], tWv[:, 1:2])
            t2 = vec.tile([NP1, NP1], f32, tag="t2")
            nc.vector.scalar_tensor_tensor(
                t2[:], p_me[:], tWv[:, 0:1], m1[:], op0=ALU.mult, op1=ALU.add)
            t2b = vec.tile([NP1, NP1], f32, tag="t2b")
            nc.vector.tensor_add(t2b[:], p_dg[:], t2[:])
            t3 = vec.tile([NP1, NP1], f32, tag="t3")
            nc.vector.tensor_sub(t3[:], t2b[:], t1[:])
            t4 = vec.tile([NP1, NP1], f32, tag="t4")
            nc.vector.tensor_mul(t4[:], t3[:], cur[:, :NP1])
            rowsum = vec.tile([NP1, 1], f32, tag="rowsum")
            nc.vector.reduce_sum(rowsum[:], t4[:], axis=AX)

            p_ged = ps_mm.tile([1, 1], f32, tag="ged")
            nc.tensor.matmul(p_ged[:], rowsum[:], tOnes[:], start=True, stop=True)
            s_out = vec.tile([1, 1], f32, tag="out_s")
            nc.scalar.copy(s_out[:], p_ged[:])
            nc.sync.dma_start(dOut[:], s_out[:])

    nc.compile()
    return nc


def _prep_inputs(adjacenceMatrix, labels, node_weighs, edge_weighs):
    """Host-side layout prep: relu/triu of the tiny weight vectors, adjacency
    binarization/one-hot, node-cost gather, bf16 packing.  All O(n^2)/O(n^3)
    compute (Sinkhorn, matmuls, reductions) runs on device."""
    import ml_dtypes

    f = np.float32
    bf = ml_dtypes.bfloat16
    n = N_G
    nw = np.maximum(np.asarray(node_weighs, dtype=f), 0.0)
    ew = np.maximum(np.asarray(edge_weighs, dtype=f), 0.0)
    iu, ju = np.triu_indices(NB_LABELS, k=1)
    NC = np.zeros((NB_LABELS, NB_LABELS), f)
    NC[iu, ju] = nw[:-1]
    NC = NC + NC.T
    nid = nw[-1]   # node insert/delete cost
    ce0 = ew[0]    # edge substitution cost (2 labels -> one off-diag value)
    eid = ew[-1]   # edge insert/delete cost

    adj = np.asarray(adjacenceMatrix)
    A1 = np.zeros((NP1, NP1), np.int64)
    A1[:n, :n] = adj[0][: n * n].reshape(n, n)
    A2 = np.zeros((NP1, NP1), np.int64)
    A2[:n, :n] = adj[1][: n * n].reshape(n, n)
    Ab1 = (A1 != 0).astype(f)
    Ab2 = (A2 != 0).astype(f)
    oh1 = [(A1 == a).astype(f) for a in (1, 2)]
    oh2 = [(A2 == a).astype(f) for a in (1, 2)]

    lab = np.asarray(labels)
    L1T = np.zeros((NB_LABELS, NP1), f)
    L1T[lab[0][:n].astype(np.int64), np.arange(n)] = 1.0
    NCL2 = np.zeros((NB_LABELS, NP1), f)
    NCL2[:, :n] = NC[:, lab[1][:n].astype(np.int64)]

    a = np.ones(NP1, f)
    a[n] = 0.0
    b = np.zeros(NP1, f)
    b[n] = 1.0
    U2 = np.stack([nid * a, nid * b])
    W2 = np.stack([b, a])

    Ipad = np.ones((NP1, NP2), f)
    Ipad[:, :NP1] = np.eye(NP1, dtype=f)

    J = np.ones((NP1, NP1), f)
    PTe = np.concatenate([0.5 * Ab1.T, 0.5 * J - Ab1.T], axis=1)
    PTc = np.concatenate([0.5 * oh1[0].T, 0.5 * oh1[1].T], axis=1)
    Qp = np.concatenate([J, Ab2, oh2[1], oh2[0]], axis=1)

    d1 = np.diag(Ab1).astype(f)
    d2 = np.diag(Ab2).astype(f)
    U4 = np.stack([
        0.5 * eid * d1,
        0.5 * eid * np.ones(NP1, f) - eid * d1,
        0.5 * ce0 * np.diag(oh1[0]).astype(f),
        0.5 * ce0 * np.diag(oh1[1]).astype(f),
    ])
    W4 = np.stack([np.ones(NP1, f), d2,
                   np.diag(oh2[1]).astype(f), np.diag(oh2[0]).astype(f)])

    wv = np.empty((NP1, 2), f)
    wv[:, 0] = eid
    wv[:, 1] = ce0

    c = np.ascontiguousarray
    return {
        "L1T": L1T, "NCL2": NCL2, "U2": c(U2), "W2": c(W2),
        "Ipad": c(Ipad.astype(bf)), "PTe": c(PTe.astype(bf)),
        "PTc": c(PTc.astype(bf)), "Qp": c(Qp.astype(bf)),
        "U4": c(U4.astype(bf)), "W4": c(W4.astype(bf)),
        "ones": np.ones((NP1, 1), f), "wv": wv,
    }


_NC = None


def _get_nc():
    global _NC
    if _NC is None:
        _NC = _build_nc()
    return _NC


def kernel(graph, adjacenceMatrix, graphCard, labels, node_weighs, edge_weighs):
    from concourse.bass_utils import run_bass_kernel_spmd

    in_map = _prep_inputs(adjacenceMatrix, labels, node_weighs, edge_weighs)
    res = run_bass_kernel_spmd(
        _get_nc(), [in_map] * N_CORES, core_ids=list(range(N_CORES)))
    return np.float32(res.results[0]["out"][0, 0])


# revision 7
# speedup vs baseline: 1.3792x; 1.0538x over previous
import sys

if "/opt/trn_rl_repo" not in sys.path:
    sys.path.insert(0, "/opt/trn_rl_repo")

import numpy as np

N_G = 90
NP1 = 91   # N_G + 1 (epsilon-padded graph order)
NP2 = 92   # NP1 + 1 (transpose output carries a sums column)
NB_LABELS = 10
SINKHORN_ITERS = 10
N_CORES = 8

# bf16 mega-blob column layout: [Ipad | PTe | PTc | Qp]
_O_IP = 0
_O_PTE = NP2
_O_PTC = _O_PTE + 2 * NP1
_O_QP = _O_PTC + 2 * NP1
_BLOB_W = _O_QP + 4 * NP1


def _build_nc():
    """Single-core Bass/Tile program, run replicated SPMD on 8 cores.

    GED of one graph pair.  The 8281x8281 cost matrix C is never formed:
    each Kronecker block is separable, so v'Fv = sum_t S.(P_t S Q_t) with
    91x91 factors.  Device pipeline (bf16 operands, f32 PSUM accumulation):
      Dg   = [L1';U2]' @ [NCL2;W2]   one K=12 matmul (costs + insdel border)
      S0   = exp(-0.5 Dg)
      S    = 10 Sinkhorn iterations; each transpose is a bf16 matmul with
             rhs [I|1] so the new frame's row sums ride in column 91
      M    = sum_t P_t S Q_t  with exact-bf16 {0,.5,1} factors split into an
             edgeInsDel group and an edge-cost group; the runtime weights are
             applied on DVE in f32 so bf16 never rounds them
      ged  = sum S.(eid*Meid + ce0*Mce0 + Dg - diagF.S)
    """
    import concourse.bass as bass
    import concourse.tile as tile
    from concourse import bacc, mybir

    f32 = mybir.dt.float32
    bf16 = mybir.dt.bfloat16
    AX = mybir.AxisListType.X
    ALU = mybir.AluOpType
    ACTF = mybir.ActivationFunctionType

    nc = bacc.Bacc(None, debug=False)

    dDgp = nc.declare_dram_parameter("dgp", [12, 2 * NP1], bf16, isOutput=False)
    dOw = nc.declare_dram_parameter("onewv", [NP1, 3], f32, isOutput=False)
    dBlob = nc.declare_dram_parameter("blob", [NP1, _BLOB_W], bf16, isOutput=False)
    dUW4 = nc.declare_dram_parameter("uw4", [4, 2 * NP1], bf16, isOutput=False)
    dOut = nc.declare_dram_parameter("out", [1, 1], f32, isOutput=True)

    with tile.TileContext(nc) as tc:
        with (
            tc.tile_pool(name="const", bufs=1) as cpool,
            tc.tile_pool(name="sk", bufs=2) as sk,
            tc.tile_pool(name="vec", bufs=1) as vec,
            tc.tile_pool(name="ps_dg", bufs=1, space=bass.MemorySpace.PSUM) as ps_dg,
            tc.tile_pool(name="ps_sk", bufs=2, space=bass.MemorySpace.PSUM) as ps_sk,
            tc.tile_pool(name="ps_mm", bufs=1, space=bass.MemorySpace.PSUM) as ps_mm,
        ):
            tDgp = cpool.tile([12, 2 * NP1], bf16)
            tOw = cpool.tile([NP1, 3], f32)
            nc.sync.dma_start(tDgp[:], dDgp[:])
            nc.sync.dma_start(tOw[:], dOw[:])
            tB = cpool.tile([NP1, _BLOB_W], bf16)
            tUW4 = cpool.tile([4, 2 * NP1], bf16)
            nc.gpsimd.dma_start(tB[:], dBlob[:])
            nc.gpsimd.dma_start(tUW4[:], dUW4[:])
            tIp = tB[:, _O_IP:_O_IP + NP2]

            # persistent reciprocal normalizer; [90] stays 1.0 (eps not normed)
            rden = vec.tile([N_G, 1], f32, tag="rden")
            rrec = vec.tile([NP1, 1], f32, tag="rrec")
            nc.vector.memset(rrec[:], 1.0)

            # ---- Dg (one matmul) and S0 = exp(-0.5 Dg) ----
            p_dg = ps_dg.tile([NP1, NP1], f32, tag="dg")
            nc.tensor.matmul(p_dg[:], tDgp[:, :NP1], tDgp[:, NP1:],
                             start=True, stop=True)
            s_cur = sk.tile([NP1, NP1], bf16, tag="s_sb")
            nc.scalar.activation(s_cur[:], p_dg[:], ACTF.Exp, scale=-0.5)
            nc.vector.reduce_sum(rden[:], s_cur[:N_G, :], axis=AX)

            # ---- Sinkhorn: 20 half-steps of normalize + fused transpose ----
            # p_h = frame_n' @ [I|1]: cols :91 = transposed frame, col 91 =
            # the new frame's row sums (only rows :90 are ever normalized).
            p_h = None
            last_T = None
            for h in range(2 * SINKHORN_ITERS):
                if h == 0:
                    nc.vector.reciprocal(rrec[:N_G, :], rden[:])
                else:
                    nc.vector.reciprocal(rrec[:N_G, :], p_h[:N_G, NP1:NP2])
                s_n = sk.tile([NP1, NP1], bf16, tag="s_sb")
                src = s_cur[:] if h == 0 else p_h[:, :NP1]
                nc.vector.tensor_scalar_mul(s_n[:], src, rrec[:])
                p_h = ps_sk.tile([NP1, NP2], f32, tag="s_ps")
                nc.tensor.matmul(p_h[:], s_n[:], tIp, start=True, stop=True)
                last_T = s_n
            cur = p_h  # V in PSUM (col 91 junk); last_T = V' in SBUF, bf16

            # ---- quadratic form, exact bf16 factors ----
            p_vb = ps_mm.tile([NP1, 4 * NP1], f32, tag="vb")
            nc.tensor.matmul(p_vb[:], last_T[:], tB[:, _O_QP:],
                             start=True, stop=True)
            s_vb = cpool.tile([NP1, 4 * NP1], bf16, tag="vb_s")
            nc.scalar.copy(s_vb[:], p_vb[:])
            p_me = ps_dg.tile([NP1, NP1], f32, tag="meid")
            nc.tensor.matmul(p_me[:], tB[:, _O_PTE:_O_PTE + NP1],
                             s_vb[:, :NP1], start=True, stop=False)
            nc.tensor.matmul(p_me[:], tB[:, _O_PTE + NP1:_O_PTE + 2 * NP1],
                             s_vb[:, NP1:2 * NP1], start=False, stop=True)
            p_mc = ps_dg.tile([NP1, NP1], f32, tag="mce")
            nc.tensor.matmul(p_mc[:], tB[:, _O_PTC:_O_PTC + NP1],
                             s_vb[:, 2 * NP1:3 * NP1], start=True, stop=False)
            nc.tensor.matmul(p_mc[:], tB[:, _O_PTC + NP1:_O_PTC + 2 * NP1],
                             s_vb[:, 3 * NP1:], start=False, stop=True)

            # diagF (host-folded weights); off critical path
            p_df = ps_mm.tile([NP1, NP1], f32, tag="df")
            nc.tensor.matmul(p_df[:], tUW4[:, :NP1], tUW4[:, NP1:],
                             start=True, stop=True)
            s_df = cpool.tile([NP1, NP1], f32, tag="df_s")
            nc.scalar.copy(s_df[:], p_df[:])

            # ---- ged = sum V.(eid*Meid + ce0*Mce0 + Dg - diagF.V) ----
            t1 = vec.tile([NP1, NP1], f32, tag="t1")
            nc.vector.tensor_mul(t1[:], s_df[:], cur[:, :NP1])
            m1 = vec.tile([NP1, NP1], f32, tag="m1")
            nc.vector.tensor_scalar_mul(m1[:], p_mc[:], tOw[:, 2:3])
            t2 = vec.tile([NP1, NP1], f32, tag="t2")
            nc.vector.scalar_tensor_tensor(
                t2[:], p_me[:], tOw[:, 1:2], m1[:], op0=ALU.mult, op1=ALU.add)
            t2b = vec.tile([NP1, NP1], f32, tag="t2b")
            nc.vector.tensor_add(t2b[:], p_dg[:], t2[:])
            t3 = vec.tile([NP1, NP1], f32, tag="t3")
            nc.vector.tensor_sub(t3[:], t2b[:], t1[:])
            t4 = vec.tile([NP1, NP1], f32, tag="t4")
            rowsum = vec.tile([NP1, 1], f32, tag="rowsum")
            nc.vector.scalar_tensor_tensor(
                t4[:], t3[:], 1.0, cur[:, :NP1],
                op0=ALU.mult, op1=ALU.mult, accum_out=rowsum[:])

            p_ged = ps_mm.tile([1, 1], f32, tag="ged")
            nc.tensor.matmul(p_ged[:], rowsum[:], tOw[:, 0:1],
                             start=True, stop=True)
            s_out = vec.tile([1, 1], f32, tag="out_s")
            nc.scalar.copy(s_out[:], p_ged[:])
            nc.sync.dma_start(dOut[:], s_out[:])

    nc.compile()
    return nc


def _prep_inputs(adjacenceMatrix, labels, node_weighs, edge_weighs):
    """Host-side layout prep: relu/triu of the tiny weight vectors, adjacency
    binarization/one-hot, node-cost gather, bf16 packing.  All O(n^2)/O(n^3)
    compute (Sinkhorn, matmuls, reductions) runs on device."""
    import ml_dtypes

    f = np.float32
    bf = ml_dtypes.bfloat16
    n = N_G
    nw = np.maximum(np.asarray(node_weighs, dtype=f), 0.0)
    ew = np.maximum(np.asarray(edge_weighs, dtype=f), 0.0)
    iu, ju = np.triu_indices(NB_LABELS, k=1)
    NC = np.zeros((NB_LABELS, NB_LABELS), f)
    NC[iu, ju] = nw[:-1]
    NC = NC + NC.T
    nid = nw[-1]   # node insert/delete cost
    ce0 = ew[0]    # edge substitution cost (2 labels -> one off-diag value)
    eid = ew[-1]   # edge insert/delete cost

    adj = np.asarray(adjacenceMatrix)
    A1 = np.zeros((NP1, NP1), np.int64)
    A1[:n, :n] = adj[0][: n * n].reshape(n, n)
    A2 = np.zeros((NP1, NP1), np.int64)
    A2[:n, :n] = adj[1][: n * n].reshape(n, n)
    Ab1 = (A1 != 0).astype(f)
    Ab2 = (A2 != 0).astype(f)
    oh1 = [(A1 == a).astype(f) for a in (1, 2)]
    oh2 = [(A2 == a).astype(f) for a in (1, 2)]

    lab = np.asarray(labels)
    L1T = np.zeros((NB_LABELS, NP1), f)
    L1T[lab[0][:n].astype(np.int64), np.arange(n)] = 1.0
    NCL2 = np.zeros((NB_LABELS, NP1), f)
    NCL2[:, :n] = NC[:, lab[1][:n].astype(np.int64)]
    a = np.ones(NP1, f)
    a[n] = 0.0
    b = np.zeros(NP1, f)
    b[n] = 1.0
    dgp = np.zeros((12, 2 * NP1), f)
    dgp[:10, :NP1] = L1T
    dgp[10, :NP1] = nid * a
    dgp[11, :NP1] = nid * b
    dgp[:10, NP1:] = NCL2
    dgp[10, NP1:] = b
    dgp[11, NP1:] = a

    J = np.ones((NP1, NP1), f)
    blob = np.empty((NP1, _BLOB_W), f)
    blob[:, _O_IP:_O_IP + NP2] = np.concatenate(
        [np.eye(NP1, dtype=f), np.ones((NP1, 1), f)], axis=1)
    blob[:, _O_PTE:_O_PTE + NP1] = 0.5 * Ab1.T
    blob[:, _O_PTE + NP1:_O_PTE + 2 * NP1] = 0.5 * J - Ab1.T
    blob[:, _O_PTC:_O_PTC + NP1] = 0.5 * oh1[0].T
    blob[:, _O_PTC + NP1:_O_PTC + 2 * NP1] = 0.5 * oh1[1].T
    blob[:, _O_QP:_O_QP + NP1] = J
    blob[:, _O_QP + NP1:_O_QP + 2 * NP1] = Ab2
    blob[:, _O_QP + 2 * NP1:_O_QP + 3 * NP1] = oh2[1]
    blob[:, _O_QP + 3 * NP1:] = oh2[0]

    d1 = np.diag(Ab1).astype(f)
    d2 = np.diag(Ab2).astype(f)
    uw4 = np.zeros((4, 2 * NP1), f)
    uw4[0, :NP1] = 0.5 * eid * d1
    uw4[1, :NP1] = 0.5 * eid * np.ones(NP1, f) - eid * d1
    uw4[2, :NP1] = 0.5 * ce0 * np.diag(oh1[0])
    uw4[3, :NP1] = 0.5 * ce0 * np.diag(oh1[1])
    uw4[0, NP1:] = 1.0
    uw4[1, NP1:] = d2
    uw4[2, NP1:] = np.diag(oh2[1])
    uw4[3, NP1:] = np.diag(oh2[0])

    onewv = np.empty((NP1, 3), f)
    onewv[:, 0] = 1.0
    onewv[:, 1] = eid
    onewv[:, 2] = ce0

    c = np.ascontiguousarray
    return {
        "dgp": c(dgp.astype(bf)), "onewv": onewv,
        "blob": c(blob.astype(bf)), "uw4": c(uw4.astype(bf)),
    }


_NC = None


def _get_nc():
    global _NC
    if _NC is None:
        _NC = _build_nc()
    return _NC


def kernel(graph, adjacenceMatrix, graphCard, labels, node_weighs, edge_weighs):
    from concourse.bass_utils import run_bass_kernel_spmd

    in_map = _prep_inputs(adjacenceMatrix, labels, node_weighs, edge_weighs)
    res = run_bass_kernel_spmd(
        _get_nc(), [in_map] * N_CORES, core_ids=list(range(N_CORES)))
    return np.float32(res.results[0]["out"][0, 0])
